# revision 1
# baseline (speedup 1.0000x reference)
"""Bahdanau additive attention on 8 Trainium2 NeuronCores.

Problem shapes (hardcoded): B=4, T=128, S=512, H=256, fp32.

Sharding: data-parallel over (batch, T-half): core c handles b = c//2,
t in [64*(c%2), 64*(c%2)+64).  Every core runs the same SPMD program on
its own shard; weights are replicated.  No collectives.

Per-core algorithm (T_loc=64, S=512, H=256), fp16 data paths with fp32
PSUM accumulation throughout:
  peT[h,s] = (Wh @ enc^T)[h,s],  pqT[h,t] = (Ws @ q^T)[h,t]
  Y[:,t] = peT + pqT[:,t]      DVE tensor_scalar_add (pq col = per-partition
                               scalar); group 0 instead fuses the add into
                               the tanh via ACT's bias operand (fast ramp)
  X = tanh(Y)                  ACT on (128 x tgs*512) tiles, staggered group
                               sizes [2,4,8,16,16,16,2] for ramp/drain
  e[t,s] = sum_h v[h]*X[h,s]   PE: shifted-column stationary v24 (v in col
                               16-jj) deposits row jj of a (16x512) PSUM
                               tile; hc-major order keeps PE unblocked
  eT assembly                  DVE copy PSUM->SBUF (fp16), then DMA xbar
                               transposes (subgroups 0-2) / PE transposes
                               (last subgroup) into eT (s-part x t-free)
  P = exp(eT - 4)              one ACT op; no max-subtraction needed since
                               |e| <= ||v||_1 ~ 12.8 and exp(e-4) fits fp16
  PTm = P * mask[s]            per-partition mul (s is the partition dim)
  [Z | c_un] = PTm^T @ [mask|enc]  one PE matmul group (col 0 = Z)
  c = c_un / Z                 DVE reciprocal + per-partition scale
  attn = tanh([q,c] @ Wout^T)  catT = [qT; cT] fp16 matmuls, ACT tanh
"""

import numpy as np

B, T, S, H = 4, 128, 512, 256
TLOC = 64          # T rows per core
NCORES = 8
TGS = 16           # t's per tanh group
NG = TLOC // TGS   # 8 groups
P = 128            # partitions
HC = H // P        # 2 h-chunks
SB = S // P        # 4 s-blocks
FC = (2 * H) // P  # 4 f-chunks of cat=[q,c]

_CACHE = {}


def build_module():
    """Build + compile the SPMD Bass module (same program for all cores)."""
    if "nc" in _CACHE:
        return _CACHE["nc"]

    try:
        import concourse.bass  # noqa: F401
    except ImportError:
        import sys
        sys.path.insert(0, "/opt/trn_rl_repo")

    import concourse.bass as bass
    import concourse.tile as tile
    import bass_rust
    from concourse import bacc, mybir

    f32 = mybir.dt.float32
    f16 = mybir.dt.float16
    f32r = mybir.dt.float32r
    AF = mybir.ActivationFunctionType

    nc = bacc.Bacc(
        "TRN2",
        target_bir_lowering=False,
        debug=False,
        enable_asserts=False,
        num_devices=NCORES,
    )

    # packed inputs (fp16): one fat DMA each instead of 21 small ones
    # pack_a: [encT0|encT1|whT0|whT1]            (128 x 1536)
    # pack_b: [qT0|qT1|wsT0|wsT1|v24_0|v24_1|ident|mask] (128 x 776)
    # pack_c: [enc0..enc3|wout0..wout3]          (128 x 2048)
    d_pa = nc.dram_tensor("pack_a", (P, 1536), f16, kind="ExternalInput").ap()
    d_pb = nc.dram_tensor("pack_b", (P, 776), f16, kind="ExternalInput").ap()
    d_pc = nc.dram_tensor("pack_c", (P, 2308), f16, kind="ExternalInput").ap()
    d_out = nc.dram_tensor("out_l", (TLOC, H), f32, kind="ExternalOutput").ap()

    with tile.TileContext(nc) as tc:
        from contextlib import ExitStack

        with ExitStack() as ctx:
            consts = ctx.enter_context(tc.tile_pool(name="consts", bufs=1))
            proj = ctx.enter_context(tc.tile_pool(name="proj", bufs=1))
            ypool = ctx.enter_context(tc.tile_pool(name="ypool", bufs=1))
            xpool = ctx.enter_context(tc.tile_pool(name="xpool", bufs=2))
            tail = ctx.enter_context(tc.tile_pool(name="tail", bufs=1))
            psA = ctx.enter_context(tc.tile_pool(name="psA", bufs=1, space="PSUM"))
            psE8 = ctx.enter_context(tc.tile_pool(name="psE8", bufs=4, space="PSUM"))
            psT = ctx.enter_context(tc.tile_pool(name="psT", bufs=3, space="PSUM"))

            # ---- load packed inputs (pack_a first: it gates the pe
            # projection and with it the whole main loop) ----
            pa = consts.tile([P, 1536], f16)
            nc.sync.dma_start(pa[:], d_pa[:, :])
            pb = consts.tile([P, 776], f16)
            nc.sync.dma_start(pb[:], d_pb[:, :])
            pc = consts.tile([P, 2308], f16)
            nc.sync.dma_start(pc[:], d_pc[:, :])
            encT_sb = [pa[:, 0:S], pa[:, S:2 * S]]
            wh_sb = [pa[:, 2 * S:2 * S + H], pa[:, 2 * S + H:2 * S + 2 * H]]
            qT_sb = [pb[:, 0:TLOC], pb[:, TLOC:2 * TLOC]]
            ws_sb = [pb[:, 128:128 + H], pb[:, 128 + H:128 + 2 * H]]
            v24_sb = [pb[:, 640:672], pb[:, 672:704]]
            ident_sb = pb[:, 704:768]
            mask_sb = pb[:, 768:772]
            menc_sb = [pc[:, sb * (H + 1):(sb + 1) * (H + 1)]
                       for sb in range(SB)]
            wout_sb = [pc[:, SB * (H + 1) + fc * H:
                          SB * (H + 1) + (fc + 1) * H] for fc in range(FC)]
            mask256_sb = pc[:, 2052:2308]
            maskf_sb = consts.tile([P, SB], f32)
            nc.vector.tensor_copy(maskf_sb[:], mask_sb)

            # ---- projections ----
            # peT[oc] (128 x 512): peT[o,s] = sum_h Wh[o,h] * encT[h,s]
            # pe_ps PSUM tiles stay live so group 0's tanh can fuse the
            # pq bias and read straight from PSUM (fast ramp).
            peT_sb = []
            pqT_sb = []
            pe_ps_l = []
            for oc in range(HC):
                pool_oc = psA if oc == 0 else psT
                pe_ps = pool_oc.tile([P, S], f32, name=f"pe_ps{oc}",
                                     tag="pe_ps" if oc == 0 else "tail")
                for kc in range(HC):
                    nc.tensor.matmul(
                        pe_ps[:],
                        lhsT=wh_sb[kc][:, oc * P:(oc + 1) * P],
                        rhs=encT_sb[kc][:],
                        start=(kc == 0),
                        stop=(kc == HC - 1),
                    )
                pe_ps_l.append(pe_ps)
                pq_ps = psT.tile([P, TLOC], f32, name=f"pq_ps{oc}", tag="tail")
                for kc in range(HC):
                    nc.tensor.matmul(
                        pq_ps[:],
                        lhsT=ws_sb[kc][:, oc * P:(oc + 1) * P],
                        rhs=qT_sb[kc][:],
                        start=(kc == 0),
                        stop=(kc == HC - 1),
                    )
                t8 = proj.tile([P, TLOC], f32, name=f"pqT_sb{oc}")
                nc.vector.tensor_copy(t8[:], pq_ps[:])
                pqT_sb.append(t8)
                t7 = proj.tile([P, S], f16, name=f"peT_sb{oc}")
                nc.vector.tensor_copy(t7[:], pe_ps[:])
                peT_sb.append(t7)

            # ---- main loop: Y = pe + pq_t ; X = tanh(Y) ; e = v^T X ----
            # e rows come from M=8 matmuls with a shifted-column stationary
            # operand: v16[hc] is (128 x 16) with v[hc] at column 8, so
            # lhsT = v16[:, 8-jj:16-jj] has v in column jj -> the matmul
            # deposits row jj = v^T X_t (zeros elsewhere) of an (8 x 512)
            # PSUM tile, accumulating over hc.  8-row tiles are dense in
            # partitions, so a single DVE copy moves each to SBUF and PE
            # mini-transposes assemble eT (s-major) for the softmax tail.
            GS = [2, 4, 8, 16, 16, 16, 2]   # staggered group sizes (sum 64)
            etiles = {}
            from concourse.tile import add_dep_helper
            pending_copies = []   # (group_emitted, copy_inst)
            eT_sb = tail.tile([P, SB * TLOC], f16)  # (128 x 256) eT cols
            e8_sbs = []
            t0g = 0
            for g, tgs in enumerate(GS):
                xs = []
                for hc in range(HC):
                    if g == 0:
                        # ramp shortcut: tanh(pe + pq_t) fused on ACT via the
                        # per-partition bias operand, reading pe from PSUM --
                        # skips the DVE add chain before the first tanh
                        x = xpool.tile([P, TGS * S], f16, name=f"x_{g}_{hc}",
                                       tag=f"x{hc}")
                        for j in range(tgs):
                            t = t0g + j
                            nc.scalar.activation(
                                x[:, j * S:(j + 1) * S],
                                pe_ps_l[hc][:],
                                AF.Tanh,
                                bias=pqT_sb[hc][:, t:t + 1],
                            )
                        xs.append(x)
                        continue
                    y = ypool.tile([P, TGS * S], f16, name=f"y_{g}_{hc}",
                                   tag=f"y{hc}")
                    for j in range(tgs):
                        t = t0g + j
                        ai = nc.vector.tensor_scalar_add(
                            y[:, j * S:(j + 1) * S],
                            peT_sb[hc][:],
                            pqT_sb[hc][:, t:t + 1],
                        )
                        if j == 0 and hc == 0:
                            # force earlier-subgroup e8 copies ahead of these
                            # adds in the DVE stream (scheduler otherwise
                            # buries the copies, starving the tail)
                            for ge, ci in list(pending_copies):
                                if ge <= g - 2:
                                    add_dep_helper(
                                        ai.ins, ci.ins, sync=False,
                                        reason="e8 copy before later adds")
                                    pending_copies.remove((ge, ci))
                    x = xpool.tile([P, TGS * S], f16, name=f"x_{g}_{hc}",
                                   tag=f"x{hc}")
                    nc.scalar.activation(x[:, 0:tgs * S], y[:, 0:tgs * S],
                                         AF.Tanh)
                    xs.append(x)
                # subgroups of 16 rows; a group smaller than 16 contributes
                # a partial subgroup, completed by later groups
                for j in range(tgs):
                    t = t0g + j
                    if t % 16 == 0:
                        u = t // 16
                        etiles[u] = psE8.tile([16, S], f32, name=f"e_{u}",
                                              tag="e_rows")
                # hc-major: all hc0 matmuls first, so the PE's in-order queue
                # is not blocked by hc1 matmuls waiting on the second tanh
                for hc in range(HC):
                    for j in range(tgs):
                        t = t0g + j
                        u, jj = t // 16, t % 16
                        nc.tensor.matmul(
                            etiles[u][:, :],
                            lhsT=v24_sb[hc][:, 16 - jj:32 - jj],
                            rhs=xs[hc][:, j * S:(j + 1) * S],
                            start=(hc == 0 and jj == 0),
                            stop=(hc == HC - 1 and jj == 15),
                            skip_group_check=True,
                        )
                for j in range(tgs):
                    t = t0g + j
                    u, jj = t // 16, t % 16
                    if jj == 15:
                        e8 = tail.tile([16, S], f16, name=f"e8_{u}",
                                       tag="e8sb", bufs=2)
                        if u < 3:
                            ci = nc.vector.tensor_copy(e8[:], etiles[u][:])
                            pending_copies.append((g, ci))
                        else:
                            nc.vector.tensor_copy(
                                e8[:, 0:S // 2], etiles[u][:, 0:S // 2])
                            nc.vector.tensor_copy(
                                e8[:, S // 2:S], etiles[u][:, S // 2:S])
                        if u < 3:
                            # DMA xbar transpose (idle queues; latency
                            # hides inside the main loop)
                            dst = eT_sb[:, u * 16:u * 16 + 16]
                            dst.ap = bass_rust.VecI64Pair(
                                [list(dst.ap[0]), [TLOC, SB], [1, 16]])
                            nc.sync.dma_start_transpose(dst, e8[:, :])
                        else:
                            # last subgroup: PE transposes (PE is idle by
                            # now; the late DMA queue would gate the tail)
                            eT3_ps = psT.tile([P, TLOC], f16, tag="tail")
                            for sb in range(SB):
                                nc.tensor.transpose(
                                    eT3_ps[:, sb * 16:(sb + 1) * 16],
                                    e8[:, sb * P:(sb + 1) * P],
                                    ident_sb[0:16, 0:16],
                                )
                            dst3 = eT_sb[:, u * 16:u * 16 + 16]
                            dst3.ap = bass_rust.VecI64Pair(
                                [list(dst3.ap[0]), [TLOC, SB], [1, 16]])
                            nc.vector.tensor_copy(dst3, eT3_ps[:])
                t0g += tgs

            # ---- softmax tail ----
            # eT_ps is (s-part x t-free); exp all chunks in one ACT op,
            # then per-partition masking per s-block chunk.
            # exp(e - 4) in fp16: |e| <= ||v||_1 ~ 12.8 so exp(e-4) < 7e3
            # stays in fp16 range; the e^-4 factor cancels in alpha = P/Z.
            negc_sb = consts.tile([P, 1], f32)
            nc.vector.memset(negc_sb[:], -4.0)
            pt_sb = tail.tile([P, SB * TLOC], f16)
            nc.scalar.activation(pt_sb[:], eT_sb[:], AF.Exp, bias=negc_sb[:, 0:1])
            ptm_all = tail.tile([P, SB * TLOC], f16)
            nc.vector.tensor_tensor(out=ptm_all[:], in0=pt_sb[:],
                                    in1=mask256_sb, op=mybir.AluOpType.mult)
            ptm_sb = [ptm_all[:, sb * TLOC:(sb + 1) * TLOC]
                      for sb in range(SB)]

            # rhs = [mask | enc] per s-block: output column 0 is the
            # softmax denominator Z, columns 1..256 the unnormalized context
            cun_ps = psT.tile([TLOC, H + 1], f32, tag="tail")
            for sb in range(SB):
                nc.tensor.matmul(
                    cun_ps[:],
                    lhsT=ptm_sb[sb][:],
                    rhs=menc_sb[sb],
                    start=(sb == 0),
                    stop=(sb == SB - 1),
                )
            r_sb = tail.tile([TLOC, 1], f32)
            nc.vector.reciprocal(r_sb[:], cun_ps[:, 0:1])
            c_sb = tail.tile([TLOC, H], f16)
            nc.vector.tensor_scalar_mul(c_sb[:], cun_ps[:, 1:H + 1], r_sb[:])

            ct_ps = psT.tile([P, 2 * TLOC], f16, tag="tail")
            for i in range(HC):
                nc.tensor.transpose(
                    ct_ps[:, i * TLOC:(i + 1) * TLOC],
                    c_sb[:, i * P:(i + 1) * P],
                    ident_sb[0:TLOC, 0:TLOC],
                )
            ct_sb = tail.tile([P, 2 * TLOC], f16)
            nc.vector.tensor_copy(ct_sb[:], ct_ps[:])

            attn_ps = psT.tile([TLOC, H], f32, tag="tail")
            cat_tiles = [
                qT_sb[0][:],
                qT_sb[1][:],
                ct_sb[:, 0:TLOC],
                ct_sb[:, TLOC:2 * TLOC],
            ]
            for fc in range(FC):
                nc.tensor.matmul(
                    attn_ps[:],
                    lhsT=cat_tiles[fc],
                    rhs=wout_sb[fc][:],
                    start=(fc == 0),
                    stop=(fc == FC - 1),
                )
            o_sb = tail.tile([TLOC, H], f32)
            nc.scalar.activation(o_sb[:], attn_ps[:], AF.Tanh)
            nc.sync.dma_start(d_out[:, :], o_sb[:])

    nc.compile()
    _CACHE["nc"] = nc
    return nc


def make_in_maps(query, encoder_outputs, src_lengths, Ws, Wh, v, Wout):
    """Host-side shard/layout prep: per-core packed fp16 inputs."""
    h16 = np.float16
    wsT = np.asarray(Ws, h16).T                      # (H, H)
    whT = np.asarray(Wh, h16).T
    woutT = np.asarray(Wout, h16).T                  # (2H, H)
    v24 = np.zeros((HC, P, 32), h16)
    for hc in range(HC):
        v24[hc, :, 16] = np.asarray(v, np.float32)[
            hc * P:(hc + 1) * P].astype(h16)
    ident = np.eye(TLOC, dtype=h16)
    sl = np.asarray(src_lengths)

    pack_a = np.zeros((NCORES, P, 1536), h16)
    pack_b = np.zeros((NCORES, P, 776), h16)
    pack_c = np.zeros((NCORES, P, 2308), h16)
    for c in range(NCORES):
        b, th = c // 2, c % 2
        t0 = th * TLOC
        encT = np.asarray(encoder_outputs[b], h16).T      # (H, S)
        enc = np.asarray(encoder_outputs[b], h16)         # (S, H)
        qT = np.asarray(query[b, t0:t0 + TLOC, :], h16).T  # (H, TLOC)
        maskc = (np.arange(S).reshape(SB, P).T
                 < int(sl[b])).astype(h16)                # (P, SB)
        for kc in range(HC):
            pack_a[c, :, kc * S:(kc + 1) * S] = encT[kc * P:(kc + 1) * P]
            pack_a[c, :, 2 * S + kc * H:2 * S + (kc + 1) * H] = \
                whT[kc * P:(kc + 1) * P]
            pack_b[c, :, kc * TLOC:(kc + 1) * TLOC] = qT[kc * P:(kc + 1) * P]
            pack_b[c, :, 128 + kc * H:128 + (kc + 1) * H] = \
                wsT[kc * P:(kc + 1) * P]
            pack_b[c, :, 640 + kc * 32:640 + (kc + 1) * 32] = v24[kc]
        pack_b[c, 0:TLOC, 704:768] = ident
        pack_b[c, :, 768:772] = maskc
        for sb in range(SB):
            pack_c[c, :, sb * (H + 1)] = maskc[:, sb]
            pack_c[c, :, sb * (H + 1) + 1:(sb + 1) * (H + 1)] = \
                enc[sb * P:(sb + 1) * P]
        for fc in range(FC):
            pack_c[c, :, SB * (H + 1) + fc * H:
                   SB * (H + 1) + (fc + 1) * H] = woutT[fc * P:(fc + 1) * P]
        for sb in range(SB):
            pack_c[c, :, 2052 + sb * TLOC:2052 + (sb + 1) * TLOC] = \
                maskc[:, sb:sb + 1]
    return [{"pack_a": np.ascontiguousarray(pack_a[c]),
             "pack_b": np.ascontiguousarray(pack_b[c]),
             "pack_c": np.ascontiguousarray(pack_c[c])}
            for c in range(NCORES)]


def kernel(query, encoder_outputs, src_lengths, Ws, Wh, v, Wout):
    from concourse.bass_utils import run_bass_kernel_spmd

    nc = build_module()
    in_maps = make_in_maps(query, encoder_outputs, src_lengths, Ws, Wh, v, Wout)
    res = run_bass_kernel_spmd(nc, in_maps, core_ids=list(range(NCORES))).results
    out = np.empty((B, T, H), np.float32)
    for c in range(NCORES):
        b, th = c // 2, c % 2
        t0 = th * TLOC
        out[b, t0:t0 + TLOC, :] = res[c]["out_l"]
    return out



# revision 2
# speedup vs baseline: 2.1099x; 2.1099x over previous
"""Bahdanau additive attention on 8 Trainium2 NeuronCores.

Problem shapes (hardcoded): B=4, T=128, S=512, H=256, fp32.

Sharding: data-parallel over (batch, T-half): core c handles b = c//2,
t in [64*(c%2), 64*(c%2)+64).  Same SPMD program on every core; weights
replicated.  No collectives.

Algorithm: the additive-attention score
    e[t,s] = sum_h v[h] * tanh(pq[t,h] + pe[s,h])
is evaluated through a separable expansion instead of materializing the
(T,S,H) tensor.  With a = tanh(pq), w = tanh(pe):
    tanh(x+y) = (a+w)/(1+a*w)  ~=  tanh(x) + sum_{j=1..J} (c0_j a^{j-1}
                                   + c1_j a^{j+1}) w^j
(banded bivariate least-squares fit; the tanh(x) term is constant over s
and drops out under softmax shift-invariance).  Each term is a rank-1
update in (t,s) contracted over h, so e becomes J*HC=20 dense matmul
passes accumulated in one PSUM bank:
    e = sum_j M_j^T @ W_j,   M_j = v o a^{j-1} (c0_j + c1_j a^2),
                             W_j = w^j.
The w-power tiles come from a product DAG split across the Scalar
(Square), Vector, and GpSimd engines; the A-side chain tiles are small
(128x128).  Masking is one extra rank-1 pass adding -30 to masked s.
Softmax tail: exp on ACT with accum_out giving the row-sum Z for free,
PE transposes for alpha^T, one matmul for the context, fp16 throughout
with fp32 PSUM accumulation.
"""

import numpy as np

B, T, S, H = 4, 128, 512, 256
TLOC = 64
NCORES = 8
P = 128
HC = H // P        # 2 h-chunks
J = 10             # expansion order

# banded fit coefficients (see fit_final.py): relerr 5.2e-3 end-to-end
C0 = [1.003741, -1.046078, 0.490639, 0.994556, 7.666589,
      -17.984771, -22.782939, 45.847261, 25.992394, -41.782779]
C1 = [-0.899934, 0.663549, -2.494447, 4.024122, 2.157494,
      -1.710386, 5.663273, -16.25282, -16.346103, 26.79806]

_CACHE = {}


def build_module():
    if "nc" in _CACHE:
        return _CACHE["nc"]

    try:
        import concourse.bass  # noqa: F401
    except ImportError:
        import sys
        sys.path.insert(0, "/opt/trn_rl_repo")

    import concourse.tile as tile
    from concourse import bacc, mybir

    f32 = mybir.dt.float32
    f16 = mybir.dt.float16
    AF = mybir.ActivationFunctionType
    ALU = mybir.AluOpType

    nc = bacc.Bacc(
        "TRN2",
        target_bir_lowering=False,
        debug=False,
        enable_asserts=False,
        num_devices=NCORES,
    )

    # packed fp16 inputs
    # pk_a: [encT (1024) | whT (512)]                       (128 x 1536)
    # pk_b: [qT (128) | wsT (512) | vbc (128) | ident (64) | mrhs (512)]
    # pk_c: [ctx enc (1024) | woutT (1024)]                 (128 x 2048)
    d_pa = nc.dram_tensor("pack_a", (P, 1536), f16, kind="ExternalInput").ap()
    d_pb = nc.dram_tensor("pack_b", (P, 1344), f16, kind="ExternalInput").ap()
    d_pc = nc.dram_tensor("pack_c", (P, 2048), f16, kind="ExternalInput").ap()
    d_out = nc.dram_tensor("out_l", (TLOC, H), f32, kind="ExternalOutput").ap()

    with tile.TileContext(nc) as tc:
        from contextlib import ExitStack

        with ExitStack() as ctx:
            consts = ctx.enter_context(tc.tile_pool(name="consts", bufs=1))
            bpow = ctx.enter_context(tc.tile_pool(name="bpow", bufs=1))
            asm = ctx.enter_context(tc.tile_pool(name="asm", bufs=1))
            tailp = ctx.enter_context(tc.tile_pool(name="tailp", bufs=1))
            psA = ctx.enter_context(tc.tile_pool(name="psA", bufs=1, space="PSUM"))
            psB = ctx.enter_context(tc.tile_pool(name="psB", bufs=1, space="PSUM"))
            psQ = ctx.enter_context(tc.tile_pool(name="psQ", bufs=1, space="PSUM"))
            psE = ctx.enter_context(tc.tile_pool(name="psE", bufs=1, space="PSUM"))
            psT = ctx.enter_context(tc.tile_pool(name="psT", bufs=3, space="PSUM"))

            pa = consts.tile([P, 1536], f16)
            nc.sync.dma_start(pa[:], d_pa[:, :])
            pb = consts.tile([P, 1344], f16)
            nc.sync.dma_start(pb[:], d_pb[:, :])
            pc = consts.tile([P, 2048], f16)
            nc.sync.dma_start(pc[:], d_pc[:, :])

            encT = [pa[:, 0:512], pa[:, 512:1024]]          # (h-chunk, s)
            wh_sb = [pa[:, 1024 + kc * H:1024 + (kc + 1) * H] for kc in range(HC)]
            qT = pb[:, 0:128]                               # [hc0 t | hc1 t]
            ws_sb = [pb[:, 128 + kc * H:128 + (kc + 1) * H] for kc in range(HC)]
            vbc = pb[:, 640:768]
            ident = pb[:, 768:832]                          # rows 0:64 = I64
            mrhs = pb[:, 832:1344]                          # (-30/128)*(1-mask)
            ctxenc = pc[:, 0:1024]                          # 4 x (128 x 256)
            wout_sb = [pc[:, 1024 + fc * H:1024 + (fc + 1) * H] for fc in range(4)]

            neg4 = consts.tile([TLOC, 1], f32)
            nc.vector.memset(neg4[:], -4.0)

            # ---- projections (PE) ----
            pe_ps = [psA.tile([P, 512], f32, name="pe_ps0"),
                     psB.tile([P, 512], f32, name="pe_ps1")]
            for oc in range(HC):
                for kc in range(HC):
                    nc.tensor.matmul(
                        pe_ps[oc][:],
                        lhsT=wh_sb[kc][:, oc * P:(oc + 1) * P],
                        rhs=encT[kc][:],
                        start=(kc == 0), stop=(kc == HC - 1),
                    )
            pq_ps = psQ.tile([P, 128], f32, name="pq_ps")
            for oc in range(HC):
                for kc in range(HC):
                    nc.tensor.matmul(
                        pq_ps[:, oc * TLOC:(oc + 1) * TLOC],
                        lhsT=ws_sb[kc][:, oc * P:(oc + 1) * P],
                        rhs=qT[:, kc * TLOC:(kc + 1) * TLOC],
                        start=(kc == 0), stop=(kc == HC - 1),
                    )

            # ---- base activations (ACT) ----
            w1 = bpow.tile([P, 1024], f16, name="w1")
            for oc in range(HC):
                nc.scalar.activation(w1[:, oc * 512:(oc + 1) * 512],
                                     pe_ps[oc][:], AF.Tanh)
            alpha = asm.tile([P, 128], f16, name="alpha")
            nc.scalar.activation(alpha[:], pq_ps[:], AF.Tanh)
            a2 = asm.tile([P, 128], f16, name="a2")
            nc.scalar.activation(a2[:], alpha[:], AF.Square)

            # ---- B-side power DAG ----
            Wt = {1: w1}
            for j in range(2, J + 1):
                Wt[j] = bpow.tile([P, 1024], f16, name=f"w{j}")
            # ACT: w2 = w^2, w6 = (w^3)^2
            nc.scalar.activation(Wt[2][:], w1[:], AF.Square)
            # GPS queue: G1,M1 first (gates pass 1), then interleaved
            At = {}
            At[0] = vbc
            for k in range(1, J):
                At[k] = asm.tile([P, 128], f16, name=f"At{k}")
            G = {}
            M = {}
            for j in range(1, J + 1):
                G[j] = asm.tile([P, 128], f16, name=f"G{j}")
                M[j] = asm.tile([P, 128], f16, name=f"M{j}")

            def mk_g(j, eng):
                eng.tensor_scalar(G[j][:], a2[:], float(C1[j - 1]),
                                  float(C0[j - 1]), ALU.mult, ALU.add)

            def mk_m(j, eng):
                eng.tensor_tensor(out=M[j][:], in0=At[j - 1][:], in1=G[j][:],
                                  op=ALU.mult)

            def mk_at(k, eng):
                src = At[k - 2] if k >= 2 else vbc
                other = a2 if k >= 2 else alpha
                eng.tensor_tensor(out=At[k][:], in0=src[:], in1=other[:],
                                  op=ALU.mult)

            def mk_w(j, a, b, eng):
                eng.tensor_tensor(out=Wt[j][:], in0=Wt[a][:], in1=Wt[b][:],
                                  op=ALU.mult)

            V = nc.vector
            GP = nc.gpsimd
            # GPS: pass-1 gate + odd-M chain + big products
            mk_g(1, GP); mk_m(1, GP)
            mk_at(2, GP); mk_g(3, GP); mk_m(3, GP)
            mk_at(4, GP)
            mk_w(4, 2, 2, GP)
            mk_g(5, GP); mk_m(5, GP)
            mk_at(6, GP); mk_g(7, GP); mk_m(7, GP)
            mk_w(8, 4, 4, GP)
            mk_at(8, GP); mk_g(9, GP); mk_m(9, GP)
            mk_w(9, 8, 1, GP)
            mk_w(10, 8, 2, GP)
            # DVE: even-M chain + big products + tail
            mk_at(1, V); mk_g(2, V); mk_m(2, V)
            mk_at(3, V); mk_g(4, V); mk_m(4, V)
            mk_w(3, 2, 1, V)
            mk_at(5, V); mk_g(6, V); mk_m(6, V)
            mk_w(5, 4, 1, V)
            mk_at(7, V); mk_g(8, V); mk_m(8, V)
            nc.scalar.activation(Wt[6][:], Wt[3][:], AF.Square)
            mk_at(9, V); mk_g(10, V); mk_m(10, V)
            mk_w(7, 4, 3, V)

            # ---- main accumulation: e = mask + sum_j M_j^T W_j ----
            ones64 = consts.tile([P, TLOC], f16)
            nc.vector.memset(ones64[:], 1.0)
            e_ps = psE.tile([TLOC, 512], f32, name="e_ps")
            nc.tensor.matmul(e_ps[:], lhsT=ones64[:], rhs=mrhs[:],
                             start=True, stop=False)
            for j in range(1, J + 1):
                for hc in range(HC):
                    nc.tensor.matmul(
                        e_ps[:],
                        lhsT=M[j][:, hc * TLOC:(hc + 1) * TLOC],
                        rhs=Wt[j][:, hc * 512:(hc + 1) * 512],
                        start=False, stop=(j == J and hc == HC - 1),
                    )

            # ---- softmax tail ----
            pt = tailp.tile([TLOC, 512], f16, name="pt")
            zacc = tailp.tile([TLOC, 1], f32, name="zacc")
            nc.scalar.activation(pt[:], e_ps[:], AF.Exp,
                                 bias=neg4[:, 0:1], accum_out=zacc[:])
            r_sb = tailp.tile([TLOC, 1], f32, name="r_sb")
            nc.vector.reciprocal(r_sb[:], zacc[:])

            ptT_ps = psT.tile([P, 256], f16, tag="tail", name="ptT_ps")
            for sb in range(4):
                nc.tensor.transpose(
                    ptT_ps[:, sb * TLOC:(sb + 1) * TLOC],
                    pt[:, sb * P:(sb + 1) * P],
                    ident[0:TLOC, 0:TLOC],
                )
            ptT = tailp.tile([P, 256], f16, name="ptT")
            nc.vector.tensor_copy(ptT[:], ptT_ps[:])

            cun_ps = psT.tile([TLOC, H], f32, tag="tail", name="cun_ps")
            for sb in range(4):
                nc.tensor.matmul(
                    cun_ps[:],
                    lhsT=ptT[:, sb * TLOC:(sb + 1) * TLOC],
                    rhs=ctxenc[:, sb * H:(sb + 1) * H],
                    start=(sb == 0), stop=(sb == 3),
                )
            c_sb = tailp.tile([TLOC, H], f16, name="c_sb")
            nc.vector.tensor_scalar_mul(c_sb[:], cun_ps[:], r_sb[:])

            ct_ps = psT.tile([P, 128], f16, tag="tail", name="ct_ps")
            for i in range(HC):
                nc.tensor.transpose(
                    ct_ps[:, i * TLOC:(i + 1) * TLOC],
                    c_sb[:, i * P:(i + 1) * P],
                    ident[0:TLOC, 0:TLOC],
                )
            ct_sb = tailp.tile([P, 128], f16, name="ct_sb")
            nc.vector.tensor_copy(ct_sb[:], ct_ps[:])

            attn_ps = psT.tile([TLOC, H], f32, tag="tail", name="attn_ps")
            cat = [qT[:, 0:TLOC], qT[:, TLOC:128],
                   ct_sb[:, 0:TLOC], ct_sb[:, TLOC:128]]
            for fc in range(4):
                nc.tensor.matmul(attn_ps[:], lhsT=cat[fc], rhs=wout_sb[fc][:],
                                 start=(fc == 0), stop=(fc == 3))
            o_sb = tailp.tile([TLOC, H], f32, name="o_sb")
            nc.scalar.activation(o_sb[:], attn_ps[:], AF.Tanh)
            nc.sync.dma_start(d_out[:, :], o_sb[:])

    nc.compile()
    _CACHE["nc"] = nc
    return nc


def make_in_maps(query, encoder_outputs, src_lengths, Ws, Wh, v, Wout):
    h16 = np.float16
    wsT = np.asarray(Ws, h16).T
    whT = np.asarray(Wh, h16).T
    woutT = np.asarray(Wout, h16).T                  # (2H, H)
    sl = np.asarray(src_lengths)
    ident = np.eye(TLOC, dtype=h16)

    pack_a = np.zeros((NCORES, P, 1536), h16)
    pack_b = np.zeros((NCORES, P, 1344), h16)
    pack_c = np.zeros((NCORES, P, 2048), h16)
    for c in range(NCORES):
        b, th = c // 2, c % 2
        t0 = th * TLOC
        encT = np.asarray(encoder_outputs[b], h16).T      # (H, S)
        enc = np.asarray(encoder_outputs[b], h16)         # (S, H)
        qTl = np.asarray(query[b, t0:t0 + TLOC, :], h16).T  # (H, TLOC)
        msk = (np.arange(S) < int(sl[b]))
        for kc in range(HC):
            pack_a[c, :, kc * 512:(kc + 1) * 512] = encT[kc * P:(kc + 1) * P]
            pack_a[c, :, 1024 + kc * H:1024 + (kc + 1) * H] = \
                whT[kc * P:(kc + 1) * P]
            pack_b[c, :, kc * TLOC:(kc + 1) * TLOC] = qTl[kc * P:(kc + 1) * P]
            pack_b[c, :, 128 + kc * H:128 + (kc + 1) * H] = \
                wsT[kc * P:(kc + 1) * P]
            pack_b[c, :, 640 + kc * TLOC:640 + (kc + 1) * TLOC] = \
                np.asarray(v, np.float32)[kc * P:(kc + 1) * P, None].astype(h16)
        pack_b[c, 0:TLOC, 768:832] = ident
        pack_b[c, :, 832:1344] = np.where(msk, 0.0, -30.0 / 128.0)[None, :]
        for sb in range(4):
            pack_c[c, :, sb * H:(sb + 1) * H] = enc[sb * P:(sb + 1) * P]
        for fc in range(4):
            pack_c[c, :, 1024 + fc * H:1024 + (fc + 1) * H] = \
                woutT[fc * P:(fc + 1) * P]
    return [{"pack_a": np.ascontiguousarray(pack_a[c]),
             "pack_b": np.ascontiguousarray(pack_b[c]),
             "pack_c": np.ascontiguousarray(pack_c[c])}
            for c in range(NCORES)]


def kernel(query, encoder_outputs, src_lengths, Ws, Wh, v, Wout):
    from concourse.bass_utils import run_bass_kernel_spmd

    nc = build_module()
    in_maps = make_in_maps(query, encoder_outputs, src_lengths, Ws, Wh, v, Wout)
    res = run_bass_kernel_spmd(nc, in_maps, core_ids=list(range(NCORES))).results
    out = np.empty((B, T, H), np.float32)
    for c in range(NCORES):
        b, th = c // 2, c % 2
        t0 = th * TLOC
        out[b, t0:t0 + TLOC, :] = res[c]["out_l"]
    return out


# revision 5
# speedup vs baseline: 2.5662x; 1.2163x over previous
"""Bahdanau additive attention on 8 Trainium2 NeuronCores.

Problem shapes (hardcoded): B=4, T=128, S=512, H=256, fp32.

Sharding: data-parallel over (batch, T-half): core c handles b = c//2,
t in [64*(c%2), 64*(c%2)+64).  Same SPMD program on every core; weights
replicated.  No collectives.

Algorithm: the additive-attention score
    e[t,s] = sum_h v[h] * tanh(pq[t,h] + pe[s,h])
is evaluated through a separable expansion instead of materializing the
(T,S,H) tensor.  With a = tanh(pq), w = tanh(pe):
    tanh(x+y) = (a+w)/(1+a*w)  ~=  tanh(x) + sum_{j=1..J} (c0_j a^{j-1}
                                   + c1_j a^{j+1}) w^j
(banded bivariate least-squares fit; the tanh(x) term is constant over s
and drops out under softmax shift-invariance).  Each term is a rank-1
update in (t,s) contracted over h, so e becomes J*HC=20 dense matmul
passes accumulated in one PSUM bank:
    e = sum_j M_j^T @ W_j,   M_j = v o a^{j-1} (c0_j + c1_j a^2),
                             W_j = w^j.
The w-power tiles come from a product DAG split across the Scalar
(Square), Vector, and GpSimd engines; the A-side chain tiles are small
(128x128).  Masking is one extra rank-1 pass adding -30 to masked s.
Softmax tail: exp on ACT with accum_out giving the row-sum Z for free,
PE transposes for alpha^T, one matmul for the context, fp16 throughout
with fp32 PSUM accumulation.
"""

import numpy as np

B, T, S, H = 4, 128, 512, 256
TLOC = 64
NCORES = 8
P = 128
HC = H // P        # 2 h-chunks
J = 10             # expansion order

# banded fit coefficients (see fit_final.py): relerr 5.2e-3 end-to-end
C0 = [1.003741, -1.046078, 0.490639, 0.994556, 7.666589,
      -17.984771, -22.782939, 45.847261, 25.992394, -41.782779]
C1 = [-0.899934, 0.663549, -2.494447, 4.024122, 2.157494,
      -1.710386, 5.663273, -16.25282, -16.346103, 26.79806]

_CACHE = {}


def build_module():
    if "nc" in _CACHE:
        return _CACHE["nc"]

    try:
        import concourse.bass  # noqa: F401
    except ImportError:
        import sys
        sys.path.insert(0, "/opt/trn_rl_repo")

    import concourse.tile as tile
    from concourse import bacc, mybir

    f32 = mybir.dt.float32
    f16 = mybir.dt.float16
    AF = mybir.ActivationFunctionType
    ALU = mybir.AluOpType

    nc = bacc.Bacc(
        "TRN2",
        target_bir_lowering=False,
        debug=False,
        enable_asserts=False,
        num_devices=NCORES,
    )

    # packed fp16 inputs
    # pk_a: [encT (1024) | whT (512)]                       (128 x 1536)
    # pk_b: [qT (128) | wsT (512) | vbc (128) | ident (64) | mrhs (512)]
    # pk_c: [ctx enc (1024) | woutT (1024)]                 (128 x 2048)
    d_pa = nc.dram_tensor("pack_a", (P, 1536), f16, kind="ExternalInput").ap()
    d_pb = nc.dram_tensor("pack_b", (P, 1344), f16, kind="ExternalInput").ap()
    d_pc = nc.dram_tensor("pack_c", (P, 2048), f16, kind="ExternalInput").ap()
    d_out = nc.dram_tensor("out_l", (TLOC, H), f32, kind="ExternalOutput").ap()

    with tile.TileContext(nc) as tc:
        from contextlib import ExitStack

        with ExitStack() as ctx:
            consts = ctx.enter_context(tc.tile_pool(name="consts", bufs=1))
            bpow = ctx.enter_context(tc.tile_pool(name="bpow", bufs=1))
            asm = ctx.enter_context(tc.tile_pool(name="asm", bufs=1))
            tailp = ctx.enter_context(tc.tile_pool(name="tailp", bufs=1))
            psA = ctx.enter_context(tc.tile_pool(name="psA", bufs=1, space="PSUM"))
            psB = ctx.enter_context(tc.tile_pool(name="psB", bufs=1, space="PSUM"))
            psQ = ctx.enter_context(tc.tile_pool(name="psQ", bufs=1, space="PSUM"))
            psE = ctx.enter_context(tc.tile_pool(name="psE", bufs=1, space="PSUM"))
            psT = ctx.enter_context(tc.tile_pool(name="psT", bufs=3, space="PSUM"))

            pb = consts.tile([P, 1344], f16)
            nc.sync.dma_start(pb[:], d_pb[:, :])
            pa = consts.tile([P, 1536], f16)
            nc.sync.dma_start(pa[:], d_pa[:, :])
            pc = consts.tile([P, 2048], f16)
            nc.sync.dma_start(pc[:], d_pc[:, :])

            encT = [pa[:, 0:512], pa[:, 512:1024]]          # (h-chunk, s)
            wh_sb = [pa[:, 1024 + kc * H:1024 + (kc + 1) * H] for kc in range(HC)]
            qT = pb[:, 0:128]                               # [hc0 t | hc1 t]
            ws_sb = [pb[:, 128 + kc * H:128 + (kc + 1) * H] for kc in range(HC)]
            vbc = pb[:, 640:768]
            ident = pb[:, 768:832]                          # rows 0:64 = I64
            mrhs = pb[:, 832:1344]                          # (-30/128)*(1-mask)
            ctxenc = pc[:, 0:1024]                          # 4 x (128 x 256)
            wout_sb = [pc[:, 1024 + fc * H:1024 + (fc + 1) * H] for fc in range(4)]

            neg4 = consts.tile([TLOC, 1], f32)
            nc.vector.memset(neg4[:], -4.0)

            ones64 = consts.tile([P, TLOC], f16)
            nc.vector.memset(ones64[:], 1.0)

            # ---- projections (PE): pq first so alpha/a2/M-chain start early
            pq_ps = psQ.tile([P, 128], f32, name="pq_ps")
            for oc in range(HC):
                for kc in range(HC):
                    nc.tensor.matmul(
                        pq_ps[:, oc * TLOC:(oc + 1) * TLOC],
                        lhsT=ws_sb[kc][:, oc * P:(oc + 1) * P],
                        rhs=qT[:, kc * TLOC:(kc + 1) * TLOC],
                        start=(kc == 0), stop=(kc == HC - 1),
                    )
            pe_ps = [psA.tile([P, 512], f32, name="pe_ps0"),
                     psB.tile([P, 512], f32, name="pe_ps1")]
            for oc in range(HC):
                for kc in range(HC):
                    nc.tensor.matmul(
                        pe_ps[oc][:],
                        lhsT=wh_sb[kc][:, oc * P:(oc + 1) * P],
                        rhs=encT[kc][:],
                        start=(kc == 0), stop=(kc == HC - 1),
                    )

            # ---- base activations (ACT): alpha/a2 first, then w halves ----
            alpha = asm.tile([P, 128], f16, name="alpha")
            nc.scalar.activation(alpha[:], pq_ps[:], AF.Tanh)
            a2 = asm.tile([P, 128], f16, name="a2")
            nc.scalar.activation(a2[:], alpha[:], AF.Square)
            w1 = bpow.tile([P, 1024], f16, name="w1")
            for oc in range(HC):
                nc.scalar.activation(w1[:, oc * 512:(oc + 1) * 512],
                                     pe_ps[oc][:], AF.Tanh)

            # ---- B-side power DAG ----
            # ACT: squares w2,w4,w8,w10; DVE: products w3,w5,w9,w6,w7 +
            # all G/At small tiles; GPS: the 10 small M multiplies only.
            Wt = {1: w1}
            for j in range(2, J + 1):
                Wt[j] = bpow.tile([P, 1024], f16, name=f"w{j}")
            At = {}
            At[0] = vbc
            for k in range(1, J):
                At[k] = asm.tile([P, 128], f16, name=f"At{k}")
            G = {}
            M = {}
            for j in range(1, J + 1):
                G[j] = asm.tile([P, 128], f16, name=f"G{j}")
                M[j] = asm.tile([P, 128], f16, name=f"M{j}")

            def mk_g(j, eng):
                eng.tensor_scalar(G[j][:], a2[:], float(C1[j - 1]),
                                  float(C0[j - 1]), ALU.mult, ALU.add)

            def mk_m(j, eng):
                eng.tensor_tensor(out=M[j][:], in0=At[j - 1][:], in1=G[j][:],
                                  op=ALU.mult)

            def mk_at(k, eng):
                src = At[k - 2] if k >= 2 else vbc
                other = a2 if k >= 2 else alpha
                eng.tensor_tensor(out=At[k][:], in0=src[:], in1=other[:],
                                  op=ALU.mult)

            def mk_w(j, a, b, eng):
                eng.tensor_tensor(out=Wt[j][:], in0=Wt[a][:], in1=Wt[b][:],
                                  op=ALU.mult)

            V = nc.vector
            GP = nc.gpsimd
            # DVE: G/At interleaved (all small, fast), then big products
            mk_g(1, V); mk_at(1, V)
            mk_g(2, V); mk_at(2, V)
            mk_g(3, V); mk_at(3, V)
            mk_g(4, V); mk_at(4, V)
            mk_g(5, V); mk_at(5, V)
            mk_g(6, V); mk_at(6, V)
            mk_g(7, V); mk_at(7, V)
            mk_g(8, V); mk_at(8, V)
            mk_g(9, V); mk_at(9, V)
            mk_g(10, V)
            # GPS: M chain (small only)
            for j in range(1, J + 1):
                mk_m(j, GP)
            # w ladder in dataflow order (program order defines deps);
            # per-engine queues: ACT w2,w4,w8,w10; DVE w3,w5,w9,w6,w7
            nc.scalar.activation(Wt[2][:], w1[:], AF.Square)
            mk_w(3, 2, 1, V)
            nc.scalar.activation(Wt[4][:], Wt[2][:], AF.Square)
            mk_w(5, 4, 1, V)
            nc.scalar.activation(Wt[8][:], Wt[4][:], AF.Square)
            mk_w(9, 5, 4, V)
            mk_w(6, 3, 3, V)
            mk_w(7, 4, 3, V)
            nc.scalar.activation(Wt[10][:], Wt[5][:], AF.Square)

            # ---- main accumulation: e = mask + sum_j M_j^T W_j ----
            e_ps = psE.tile([TLOC, 512], f32, name="e_ps")
            nc.tensor.matmul(e_ps[:], lhsT=ones64[:], rhs=mrhs[:],
                             start=True, stop=False)
            pass_order = [1, 2, 3, 4, 5, 8, 9, 6, 10, 7]
            for n, j in enumerate(pass_order):
                for hc in range(HC):
                    nc.tensor.matmul(
                        e_ps[:],
                        lhsT=M[j][:, hc * TLOC:(hc + 1) * TLOC],
                        rhs=Wt[j][:, hc * 512:(hc + 1) * 512],
                        start=False,
                        stop=(n == len(pass_order) - 1 and hc == HC - 1),
                    )

            # ---- softmax tail ----
            pt = tailp.tile([TLOC, 512], f16, name="pt")
            zacc = tailp.tile([TLOC, 1], f32, name="zacc")
            nc.scalar.activation(pt[:], e_ps[:], AF.Exp,
                                 bias=neg4[:, 0:1], accum_out=zacc[:])
            r_sb = tailp.tile([TLOC, 1], f32, name="r_sb")
            nc.vector.reciprocal(r_sb[:], zacc[:])

            ptT_ps = psT.tile([P, 256], f16, tag="tail", name="ptT_ps")
            for sb in range(4):
                nc.tensor.transpose(
                    ptT_ps[:, sb * TLOC:(sb + 1) * TLOC],
                    pt[:, sb * P:(sb + 1) * P],
                    ident[0:TLOC, 0:TLOC],
                )
            ptT = tailp.tile([P, 256], f16, name="ptT")
            for sb in range(4):
                nc.vector.tensor_copy(ptT[:, sb * TLOC:(sb + 1) * TLOC],
                                      ptT_ps[:, sb * TLOC:(sb + 1) * TLOC])

            cun_ps = psT.tile([TLOC, H], f32, tag="tail", name="cun_ps")
            for sb in range(4):
                nc.tensor.matmul(
                    cun_ps[:],
                    lhsT=ptT[:, sb * TLOC:(sb + 1) * TLOC],
                    rhs=ctxenc[:, sb * H:(sb + 1) * H],
                    start=(sb == 0), stop=(sb == 3),
                )
            c_sb = tailp.tile([TLOC, H], f16, name="c_sb")
            nc.vector.tensor_scalar_mul(c_sb[:], cun_ps[:], r_sb[:])

            ct_ps = psT.tile([P, 128], f16, tag="tail", name="ct_ps")
            for i in range(HC):
                nc.tensor.transpose(
                    ct_ps[:, i * TLOC:(i + 1) * TLOC],
                    c_sb[:, i * P:(i + 1) * P],
                    ident[0:TLOC, 0:TLOC],
                )
            ct_sb = tailp.tile([P, 128], f16, name="ct_sb")
            nc.vector.tensor_copy(ct_sb[:], ct_ps[:])

            attn_ps = psT.tile([TLOC, H], f32, tag="tail", name="attn_ps")
            cat = [qT[:, 0:TLOC], qT[:, TLOC:128],
                   ct_sb[:, 0:TLOC], ct_sb[:, TLOC:128]]
            for fc in range(4):
                nc.tensor.matmul(attn_ps[:], lhsT=cat[fc], rhs=wout_sb[fc][:],
                                 start=(fc == 0), stop=(fc == 3))
            o_sb = tailp.tile([TLOC, H], f32, name="o_sb")
            nc.scalar.activation(o_sb[:], attn_ps[:], AF.Tanh)
            nc.sync.dma_start(d_out[:, :], o_sb[:])

    nc.compile()
    _CACHE["nc"] = nc
    return nc


def make_in_maps(query, encoder_outputs, src_lengths, Ws, Wh, v, Wout):
    h16 = np.float16
    wsT = np.asarray(Ws, h16).T
    whT = np.asarray(Wh, h16).T
    woutT = np.asarray(Wout, h16).T                  # (2H, H)
    sl = np.asarray(src_lengths)
    ident = np.eye(TLOC, dtype=h16)

    pack_a = np.zeros((NCORES, P, 1536), h16)
    pack_b = np.zeros((NCORES, P, 1344), h16)
    pack_c = np.zeros((NCORES, P, 2048), h16)
    for c in range(NCORES):
        b, th = c // 2, c % 2
        t0 = th * TLOC
        encT = np.asarray(encoder_outputs[b], h16).T      # (H, S)
        enc = np.asarray(encoder_outputs[b], h16)         # (S, H)
        qTl = np.asarray(query[b, t0:t0 + TLOC, :], h16).T  # (H, TLOC)
        msk = (np.arange(S) < int(sl[b]))
        for kc in range(HC):
            pack_a[c, :, kc * 512:(kc + 1) * 512] = encT[kc * P:(kc + 1) * P]
            pack_a[c, :, 1024 + kc * H:1024 + (kc + 1) * H] = \
                whT[kc * P:(kc + 1) * P]
            pack_b[c, :, kc * TLOC:(kc + 1) * TLOC] = qTl[kc * P:(kc + 1) * P]
            pack_b[c, :, 128 + kc * H:128 + (kc + 1) * H] = \
                wsT[kc * P:(kc + 1) * P]
            pack_b[c, :, 640 + kc * TLOC:640 + (kc + 1) * TLOC] = \
                np.asarray(v, np.float32)[kc * P:(kc + 1) * P, None].astype(h16)
        pack_b[c, 0:TLOC, 768:832] = ident
        pack_b[c, :, 832:1344] = np.where(msk, 0.0, -30.0 / 128.0)[None, :]
        for sb in range(4):
            pack_c[c, :, sb * H:(sb + 1) * H] = enc[sb * P:(sb + 1) * P]
        for fc in range(4):
            pack_c[c, :, 1024 + fc * H:1024 + (fc + 1) * H] = \
                woutT[fc * P:(fc + 1) * P]
    return [{"pack_a": np.ascontiguousarray(pack_a[c]),
             "pack_b": np.ascontiguousarray(pack_b[c]),
             "pack_c": np.ascontiguousarray(pack_c[c])}
            for c in range(NCORES)]


def kernel(query, encoder_outputs, src_lengths, Ws, Wh, v, Wout):
    from concourse.bass_utils import run_bass_kernel_spmd

    nc = build_module()
    in_maps = make_in_maps(query, encoder_outputs, src_lengths, Ws, Wh, v, Wout)
    res = run_bass_kernel_spmd(nc, in_maps, core_ids=list(range(NCORES))).results
    out = np.empty((B, T, H), np.float32)
    for c in range(NCORES):
        b, th = c // 2, c % 2
        t0 = th * TLOC
        out[b, t0:t0 + TLOC, :] = res[c]["out_l"]
    return out


# revision 6
# speedup vs baseline: 2.7337x; 1.0653x over previous
"""Bahdanau additive attention on 8 Trainium2 NeuronCores.

Problem shapes (hardcoded): B=4, T=128, S=512, H=256, fp32.

Sharding: data-parallel over (batch, T-half): core c handles b = c//2,
t in [64*(c%2), 64*(c%2)+64).  Same SPMD program on every core; weights
replicated.  No collectives.

Algorithm: the additive-attention score
    e[t,s] = sum_h v[h] * tanh(pq[t,h] + pe[s,h])
is evaluated through a separable expansion instead of materializing the
(T,S,H) tensor.  With a = tanh(pq), w = tanh(pe):
    tanh(x+y) = (a+w)/(1+a*w)  ~=  tanh(x) + sum_{j=1..J} (c0_j a^{j-1}
                                   + c1_j a^{j+1}) w^j
(banded bivariate least-squares fit; the tanh(x) term is constant over s
and drops out under softmax shift-invariance).  Each term is a rank-1
update in (t,s) contracted over h, so e becomes J*HC=20 dense matmul
passes accumulated in one PSUM bank:
    e = sum_j M_j^T @ W_j,   M_j = v o a^{j-1} (c0_j + c1_j a^2),
                             W_j = w^j.
The w-power tiles come from a product DAG split across the Scalar
(Square), Vector, and GpSimd engines; the A-side chain tiles are small
(128x128).  Masking is one extra rank-1 pass adding -30 to masked s.
Softmax tail: exp on ACT with accum_out giving the row-sum Z for free,
PE transposes for alpha^T, one matmul for the context, fp16 throughout
with fp32 PSUM accumulation.
"""

import numpy as np

B, T, S, H = 4, 128, 512, 256
TLOC = 64
NCORES = 8
P = 128
HC = H // P        # 2 h-chunks
J = 9              # expansion order

# banded fit coefficients (see fit_final.py): relerr 6.1e-3 end-to-end
C0 = [1.003813, -0.94063, 0.484471, -2.849085, 7.735563,
      9.050273, -22.98368, -14.958482, 26.153909]
C1 = [-0.899529, 1.332728, -2.497427, -0.527733, 2.126473,
      -1.194412, 5.811916, 9.643296, -16.48554]

_CACHE = {}


def build_module():
    if "nc" in _CACHE:
        return _CACHE["nc"]

    try:
        import concourse.bass  # noqa: F401
    except ImportError:
        import sys
        sys.path.insert(0, "/opt/trn_rl_repo")

    import concourse.tile as tile
    from concourse import bacc, mybir

    f32 = mybir.dt.float32
    f16 = mybir.dt.float16
    AF = mybir.ActivationFunctionType
    ALU = mybir.AluOpType

    nc = bacc.Bacc(
        "TRN2",
        target_bir_lowering=False,
        debug=False,
        enable_asserts=False,
        num_devices=NCORES,
    )

    # packed fp16 inputs
    # pk_b: [qT (128) | wsT (512) | vbc (128) | ident (64)] (128 x 832)
    # pk_a: [encT (1024) | whT (512)]                       (128 x 1536)
    # pk_c: [ctx enc (1024) | woutT (1024) | mrhs (512)]    (128 x 2560)
    d_pa = nc.dram_tensor("pack_a", (P, 1536), f16, kind="ExternalInput").ap()
    d_pb = nc.dram_tensor("pack_b", (P, 832), f16, kind="ExternalInput").ap()
    d_pc = nc.dram_tensor("pack_c", (P, 2560), f16, kind="ExternalInput").ap()
    d_out = nc.dram_tensor("out_l", (TLOC, H), f32, kind="ExternalOutput").ap()

    with tile.TileContext(nc) as tc:
        from contextlib import ExitStack

        with ExitStack() as ctx:
            consts = ctx.enter_context(tc.tile_pool(name="consts", bufs=1))
            bpow = ctx.enter_context(tc.tile_pool(name="bpow", bufs=1))
            asm = ctx.enter_context(tc.tile_pool(name="asm", bufs=1))
            tailp = ctx.enter_context(tc.tile_pool(name="tailp", bufs=1))
            psA = ctx.enter_context(tc.tile_pool(name="psA", bufs=1, space="PSUM"))
            psB = ctx.enter_context(tc.tile_pool(name="psB", bufs=1, space="PSUM"))
            psQ = ctx.enter_context(tc.tile_pool(name="psQ", bufs=1, space="PSUM"))
            psE = ctx.enter_context(tc.tile_pool(name="psE", bufs=1, space="PSUM"))
            psT = ctx.enter_context(tc.tile_pool(name="psT", bufs=3, space="PSUM"))

            pb = consts.tile([P, 832], f16)
            nc.sync.dma_start(pb[:], d_pb[:, :])
            pa = consts.tile([P, 1536], f16)
            nc.sync.dma_start(pa[:], d_pa[:, :])
            pc = consts.tile([P, 2560], f16)
            nc.sync.dma_start(pc[:], d_pc[:, :])

            encT = [pa[:, 0:512], pa[:, 512:1024]]          # (h-chunk, s)
            wh_sb = [pa[:, 1024 + kc * H:1024 + (kc + 1) * H] for kc in range(HC)]
            qT = pb[:, 0:128]                               # [hc0 t | hc1 t]
            ws_sb = [pb[:, 128 + kc * H:128 + (kc + 1) * H] for kc in range(HC)]
            vbc = pb[:, 640:768]
            ident = pb[:, 768:832]                          # rows 0:64 = I64
            ctxenc = pc[:, 0:1024]                          # 4 x (128 x 256)
            wout_sb = [pc[:, 1024 + fc * H:1024 + (fc + 1) * H] for fc in range(4)]
            mrhs = pc[:, 2048:2560]                         # (-30/128)*(1-mask)

            neg4 = consts.tile([TLOC, 1], f32)
            nc.vector.memset(neg4[:], -4.0)

            ones64 = consts.tile([P, TLOC], f16)
            nc.vector.memset(ones64[:], 1.0)

            # ---- projections (PE): pq first so alpha/a2/M-chain start early
            pq_ps = psQ.tile([P, 128], f32, name="pq_ps")
            for oc in range(HC):
                for kc in range(HC):
                    nc.tensor.matmul(
                        pq_ps[:, oc * TLOC:(oc + 1) * TLOC],
                        lhsT=ws_sb[kc][:, oc * P:(oc + 1) * P],
                        rhs=qT[:, kc * TLOC:(kc + 1) * TLOC],
                        start=(kc == 0), stop=(kc == HC - 1),
                    )
            pe_ps = [psA.tile([P, 512], f32, name="pe_ps0"),
                     psB.tile([P, 512], f32, name="pe_ps1")]
            for oc in range(HC):
                for kc in range(HC):
                    nc.tensor.matmul(
                        pe_ps[oc][:],
                        lhsT=wh_sb[kc][:, oc * P:(oc + 1) * P],
                        rhs=encT[kc][:],
                        start=(kc == 0), stop=(kc == HC - 1),
                    )

            # ---- base activations (ACT): alpha/a2 first, then w halves ----
            alpha = asm.tile([P, 128], f16, name="alpha")
            a2 = asm.tile([P, 128], f16, name="a2")
            with tc.high_priority():
                nc.scalar.activation(alpha[:], pq_ps[:], AF.Tanh)
                nc.scalar.activation(a2[:], alpha[:], AF.Square)
            w1 = bpow.tile([P, 1024], f16, name="w1")
            for oc in range(HC):
                nc.scalar.activation(w1[:, oc * 512:(oc + 1) * 512],
                                     pe_ps[oc][:], AF.Tanh)

            # ---- B-side power DAG ----
            # ACT: squares w2,w4,w8,w10; DVE: products w3,w5,w9,w6,w7 +
            # all G/At small tiles; GPS: the 10 small M multiplies only.
            Wt = {1: w1}
            for j in range(2, J + 1):
                Wt[j] = bpow.tile([P, 1024], f16, name=f"w{j}")
            At = {}
            At[0] = vbc
            for k in range(1, J):
                At[k] = asm.tile([P, 128], f16, name=f"At{k}")
            G = {}
            M = {}
            for j in range(1, J + 1):
                G[j] = asm.tile([P, 128], f16, name=f"G{j}")
                M[j] = asm.tile([P, 128], f16, name=f"M{j}")

            def mk_g(j, eng):
                eng.tensor_scalar(G[j][:], a2[:], float(C1[j - 1]),
                                  float(C0[j - 1]), ALU.mult, ALU.add)

            def mk_m(j, eng):
                eng.tensor_tensor(out=M[j][:], in0=At[j - 1][:], in1=G[j][:],
                                  op=ALU.mult)

            def mk_at(k, eng):
                src = At[k - 2] if k >= 2 else vbc
                other = a2 if k >= 2 else alpha
                eng.tensor_tensor(out=At[k][:], in0=src[:], in1=other[:],
                                  op=ALU.mult)

            def mk_w(j, a, b, eng):
                eng.tensor_tensor(out=Wt[j][:], in0=Wt[a][:], in1=Wt[b][:],
                                  op=ALU.mult)

            V = nc.vector
            GP = nc.gpsimd
            # DVE: G/At interleaved (all small, fast), then big products
            mk_g(1, V); mk_at(1, V)
            mk_g(2, V); mk_at(2, V)
            mk_g(3, V); mk_at(3, V)
            mk_g(4, V); mk_at(4, V)
            mk_g(5, V); mk_at(5, V)
            mk_g(6, V); mk_at(6, V)
            mk_g(7, V); mk_at(7, V)
            mk_g(8, V); mk_at(8, V)
            mk_g(9, V)
            # GPS: M chain (small only)
            for j in range(1, J + 1):
                mk_m(j, GP)
            # w ladder in dataflow order (program order defines deps);
            # per-engine queues: ACT w2,w4,w8; DVE w3,w5,w9,w6,w7
            nc.scalar.activation(Wt[2][:], w1[:], AF.Square)
            mk_w(3, 2, 1, V)
            nc.scalar.activation(Wt[4][:], Wt[2][:], AF.Square)
            mk_w(5, 4, 1, V)
            nc.scalar.activation(Wt[8][:], Wt[4][:], AF.Square)
            mk_w(9, 5, 4, V)
            mk_w(6, 3, 3, V)
            mk_w(7, 4, 3, V)

            # ---- main accumulation: e = sum_j M_j^T W_j + mask ----
            e_ps = psE.tile([TLOC, 512], f32, name="e_ps")
            pass_order = [1, 2, 3, 4, 5, 8, 9, 6, 7]
            for n, j in enumerate(pass_order):
                for hc in range(HC):
                    nc.tensor.matmul(
                        e_ps[:],
                        lhsT=M[j][:, hc * TLOC:(hc + 1) * TLOC],
                        rhs=Wt[j][:, hc * 512:(hc + 1) * 512],
                        start=(n == 0 and hc == 0), stop=False,
                    )
            nc.tensor.matmul(e_ps[:], lhsT=ones64[:], rhs=mrhs[:],
                             start=False, stop=True)

            # ---- softmax tail ----
            pt = tailp.tile([TLOC, 512], f16, name="pt")
            zacc = tailp.tile([TLOC, 1], f32, name="zacc")
            nc.scalar.activation(pt[:], e_ps[:], AF.Exp,
                                 bias=neg4[:, 0:1], accum_out=zacc[:])
            r_sb = tailp.tile([TLOC, 1], f32, name="r_sb")
            nc.vector.reciprocal(r_sb[:], zacc[:])

            ptT_ps = psT.tile([P, 256], f16, tag="tail", name="ptT_ps")
            for sb in range(4):
                nc.tensor.transpose(
                    ptT_ps[:, sb * TLOC:(sb + 1) * TLOC],
                    pt[:, sb * P:(sb + 1) * P],
                    ident[0:TLOC, 0:TLOC],
                )
            ptT = tailp.tile([P, 256], f16, name="ptT")
            for sb in range(4):
                nc.vector.tensor_copy(ptT[:, sb * TLOC:(sb + 1) * TLOC],
                                      ptT_ps[:, sb * TLOC:(sb + 1) * TLOC])

            cun_ps = psT.tile([TLOC, H], f32, tag="tail", name="cun_ps")
            for sb in range(4):
                nc.tensor.matmul(
                    cun_ps[:],
                    lhsT=ptT[:, sb * TLOC:(sb + 1) * TLOC],
                    rhs=ctxenc[:, sb * H:(sb + 1) * H],
                    start=(sb == 0), stop=(sb == 3),
                )
            c_sb = tailp.tile([TLOC, H], f16, name="c_sb")
            nc.vector.tensor_scalar_mul(c_sb[:], cun_ps[:], r_sb[:])

            ct_ps = psT.tile([P, 128], f16, tag="tail", name="ct_ps")
            for i in range(HC):
                nc.tensor.transpose(
                    ct_ps[:, i * TLOC:(i + 1) * TLOC],
                    c_sb[:, i * P:(i + 1) * P],
                    ident[0:TLOC, 0:TLOC],
                )
            ct_sb = tailp.tile([P, 128], f16, name="ct_sb")
            nc.vector.tensor_copy(ct_sb[:], ct_ps[:])

            attn_ps = psT.tile([TLOC, H], f32, tag="tail", name="attn_ps")
            cat = [qT[:, 0:TLOC], qT[:, TLOC:128],
                   ct_sb[:, 0:TLOC], ct_sb[:, TLOC:128]]
            for fc in range(4):
                nc.tensor.matmul(attn_ps[:], lhsT=cat[fc], rhs=wout_sb[fc][:],
                                 start=(fc == 0), stop=(fc == 3))
            o_sb = tailp.tile([TLOC, H], f32, name="o_sb")
            nc.scalar.activation(o_sb[:], attn_ps[:], AF.Tanh)
            nc.sync.dma_start(d_out[:, :], o_sb[:])

    nc.compile()
    _CACHE["nc"] = nc
    return nc


def make_in_maps(query, encoder_outputs, src_lengths, Ws, Wh, v, Wout):
    h16 = np.float16
    wsT = np.asarray(Ws, h16).T
    whT = np.asarray(Wh, h16).T
    woutT = np.asarray(Wout, h16).T                  # (2H, H)
    sl = np.asarray(src_lengths)
    ident = np.eye(TLOC, dtype=h16)

    pack_a = np.zeros((NCORES, P, 1536), h16)
    pack_b = np.zeros((NCORES, P, 832), h16)
    pack_c = np.zeros((NCORES, P, 2560), h16)
    for c in range(NCORES):
        b, th = c // 2, c % 2
        t0 = th * TLOC
        encT = np.asarray(encoder_outputs[b], h16).T      # (H, S)
        enc = np.asarray(encoder_outputs[b], h16)         # (S, H)
        qTl = np.asarray(query[b, t0:t0 + TLOC, :], h16).T  # (H, TLOC)
        msk = (np.arange(S) < int(sl[b]))
        for kc in range(HC):
            pack_a[c, :, kc * 512:(kc + 1) * 512] = encT[kc * P:(kc + 1) * P]
            pack_a[c, :, 1024 + kc * H:1024 + (kc + 1) * H] = \
                whT[kc * P:(kc + 1) * P]
            pack_b[c, :, kc * TLOC:(kc + 1) * TLOC] = qTl[kc * P:(kc + 1) * P]
            pack_b[c, :, 128 + kc * H:128 + (kc + 1) * H] = \
                wsT[kc * P:(kc + 1) * P]
            pack_b[c, :, 640 + kc * TLOC:640 + (kc + 1) * TLOC] = \
                np.asarray(v, np.float32)[kc * P:(kc + 1) * P, None].astype(h16)
        pack_b[c, 0:TLOC, 768:832] = ident
        pack_c[c, :, 2048:2560] = np.where(msk, 0.0, -30.0 / 128.0)[None, :]
        for sb in range(4):
            pack_c[c, :, sb * H:(sb + 1) * H] = enc[sb * P:(sb + 1) * P]
        for fc in range(4):
            pack_c[c, :, 1024 + fc * H:1024 + (fc + 1) * H] = \
                woutT[fc * P:(fc + 1) * P]
    return [{"pack_a": np.ascontiguousarray(pack_a[c]),
             "pack_b": np.ascontiguousarray(pack_b[c]),
             "pack_c": np.ascontiguousarray(pack_c[c])}
            for c in range(NCORES)]


def kernel(query, encoder_outputs, src_lengths, Ws, Wh, v, Wout):
    from concourse.bass_utils import run_bass_kernel_spmd

    nc = build_module()
    in_maps = make_in_maps(query, encoder_outputs, src_lengths, Ws, Wh, v, Wout)
    res = run_bass_kernel_spmd(nc, in_maps, core_ids=list(range(NCORES))).results
    out = np.empty((B, T, H), np.float32)
    for c in range(NCORES):
        b, th = c // 2, c % 2
        t0 = th * TLOC
        out[b, t0:t0 + TLOC, :] = res[c]["out_l"]
    return out
